# revision 33
# baseline (speedup 1.0000x reference)
"""Trainium2 Bass kernel for gated multi-head attention + residual + LayerNorm.

Problem (nn_CNP_5669356834854):
    B=2, L=2048, D=1024, H=16, DK=DV=64
    Q = q@wq.T+bq; K = k@wk.T+bk; V = v@wv.T+bv   (per-head split)
    attn = softmax((Q K^T / sqrt(DK)) * k_gate  [masked])
    out = LayerNorm(attn @ V @ wo.T + bo + q)

Sharding: 8 cores = (batch b) x (head-group hg, 4 heads).  L1 computes
UNNORMALIZED per-head attention outputs O^T plus softmax denominators
(ones-augmented V).  The host normalizes O (cheap elementwise) while
resharding; L2 shards (batch, 512-row chunk): output projection +
residual + LayerNorm.

L1 v2 (vs baseline): DVE gate-multiply is the roofline (16.8M elem/core
@ ~1.04ns).  Changes to pull the other engines below it:
  - gate staged as uint8 (g*255); the 1/255 folds into the ACT exp
    scale.  Halves gate DMA (33.5 -> 16.8 MB/core) at ~0.1% error.
  - softmax weights P written by ACT exp directly as fp8e5 (e5m2 range
    covers the e^17 logit spread; EXP_BIAS=-1 keeps max p ~1.6e4).
  - V path in fp8e4: x_v / wv (x16) / Vaug storage; V-projection and
    the O accumulation run as DoubleRow fp8 matmuls (2 contraction
    tiles per instruction, 2x PE throughput).
  - finer projection granules (512 cols) + DMA queue interleaving so
    the first S-tick starts ~12us in; V-projs and mt1 projections are
    spread between early ticks; O-matmuls lag fronts by 28 ticks early
    (V arrives late) and catch up to lag 8 past tick 64.
"""

import numpy as np
import ml_dtypes

import concourse.bacc as bacc
import concourse.tile as tile
from concourse import mybir
from concourse.bass_utils import run_bass_kernel_spmd

B, L, D, H, DK, DV = 2, 2048, 1024, 16, 64, 64
EPS = 1e-5
NCORE = 8
HPC = 4  # heads per core
NKC = D // 128  # 8 contraction chunks
NLKT = 16  # lk tiles
CH = 512  # L2 row-chunk per core
MPC = HPC * DK  # 256 projected rows per core
QC = 512  # lq per block
EXP_BIAS = -1.0
C_V = 16.0  # V scaled by 16 into fp8e4; host divides it back out

F32 = mybir.dt.float32
BF16 = mybir.dt.bfloat16
FP8E4 = mybir.dt.float8e4
FP8E5 = mybir.dt.float8e5
U8 = mybir.dt.uint8
AF = mybir.ActivationFunctionType
DR = mybir.MatmulPerfMode.DoubleRow

NPBF16 = ml_dtypes.bfloat16
NPE4 = ml_dtypes.float8_e4m3

N_WARM = 8
NG = 8 * NLKT  # 128 global ticks
G_AHEAD = 8  # gate slab DMA lookahead (ticks)


def _bf(x):
    return np.ascontiguousarray(x).astype(NPBF16)


def _e4(x):
    return np.ascontiguousarray(x).astype(NPE4)


def _kc_layout(a):
    """[D, N] -> [128, NKC, N] with row r = kc*128+p  ->  [p, kc, :]."""
    d, n = a.shape
    assert d == NKC * 128
    return np.ascontiguousarray(a.reshape(NKC, 128, n).transpose(1, 0, 2))


def _lag(t):
    return 28 if t < 64 else max(8, 28 - (t - 64))


def build_l1(masked: bool, use_bq: bool, use_bk: bool, use_bv: bool):
    nc = bacc.Bacc("TRN2", target_bir_lowering=False)

    qT = nc.declare_dram_parameter("qT", [128, NKC, L], BF16, isOutput=False)
    kT = nc.declare_dram_parameter("kT", [128, NKC, L], BF16, isOutput=False)
    vT8 = nc.declare_dram_parameter("vT8", [128, NKC, L], FP8E4, isOutput=False)
    wqT = nc.declare_dram_parameter("wqT", [128, NKC, MPC], BF16, isOutput=False)
    wkT = nc.declare_dram_parameter("wkT", [128, NKC, MPC], BF16, isOutput=False)
    wvT8 = nc.declare_dram_parameter("wvT8", [128, NKC, MPC], FP8E4, isOutput=False)
    # host-packed gate, 4 ticks per slab:
    # gPK[pr, qc, kp, p, k4, hp*512 + i] = round(255*g) for lkt = 4*kp+k4
    gPK = nc.declare_dram_parameter(
        "gPK", [2, 4, 4, 128, 4 * 2 * QC], U8, isOutput=False
    )
    if use_bq:
        bqP = nc.declare_dram_parameter("bqP", [128, 2], F32, isOutput=False)
    if use_bk:
        bkP = nc.declare_dram_parameter("bkP", [128, 2], F32, isOutput=False)
    if use_bv:
        bvR = nc.declare_dram_parameter("bvR", [1, MPC], F32, isOutput=False)
    if masked:
        mbT = nc.declare_dram_parameter("mbT", [L, L], BF16, isOutput=False)
    # unnormalized O (rows 0:64 per hp, scaled by C_V) + denominator (row 64)
    oU = nc.declare_dram_parameter("oU", [2, 4, 65, 2, QC], BF16, isOutput=True)

    with tile.TileContext(nc) as tc:
        with (
            tc.tile_pool(name="xb", bufs=2) as xb,
            tc.tile_pool(name="xv", bufs=1) as xvp,
            tc.tile_pool(name="ws", bufs=1) as ws,
            tc.tile_pool(name="qk", bufs=1) as qk,
            tc.tile_pool(name="gp", bufs=8) as gp,
            tc.tile_pool(name="tp", bufs=2) as tp,
            tc.tile_pool(name="pp", bufs=8) as pp,
            tc.tile_pool(name="op", bufs=2) as opl,
            tc.tile_pool(name="ps_s", bufs=3, space="PSUM") as ps_s,
            tc.tile_pool(name="ps_o", bufs=2, space="PSUM") as ps_o,
        ):
            # ---- persistent tiles ----
            wk_sb = ws.tile([128, NKC, MPC], BF16, tag="wk")
            wq_sb = ws.tile([128, NKC, MPC], BF16, tag="wq")
            wv_sb = ws.tile([128, NKC, MPC], FP8E4, tag="wv")
            x_k = xb.tile([128, NKC, L], BF16, tag="x", name="x_k")
            x_q = xb.tile([128, NKC, L], BF16, tag="x", name="x_q")
            x_v = xvp.tile([128, NKC, L], FP8E4, tag="xv")
            QT = qk.tile([128, 2, L], BF16, tag="qt")
            KT = qk.tile([128, 2, L], BF16, tag="kt")
            Vaug = qk.tile([128, NLKT, HPC, 128], FP8E4, tag="va")
            nc.vector.memset(Vaug[:, :, :, 64:128], 1.0)
            ebias = ws.tile([128, 1], F32, tag="eb")
            nc.vector.memset(ebias, EXP_BIAS)

            # ---- DMA: gpsimd (SWDGE) queue: weights wq, wv, later oU ----
            nc.gpsimd.dma_start(out=wq_sb, in_=wqT[:, :, :])
            nc.gpsimd.dma_start(out=wv_sb, in_=wvT8[:, :, :])

            # ---- DMA: sync (HWDGE) queue, interleaved for earliest start --
            gate_slabs = {}  # slab group sg -> tile [128, 4, 1024]

            def issue_gates(sg):
                """One DMA for ticks 4sg .. 4sg+3 (one (pr,qc,kp) slab)."""
                blk, kp = (4 * sg) // NLKT, ((4 * sg) % NLKT) // 4
                pr, qc = blk // 4, blk % 4
                g = gp.tile([128, 4, 2 * QC], U8, tag="g", name=f"g{sg}")
                nc.sync.dma_start(out=g, in_=gPK[pr, qc, kp, :, :])
                gate_slabs[sg] = g

            def xq_quarter(i):
                nc.sync.dma_start(
                    out=x_q[:, :, i * 512 : (i + 1) * 512],
                    in_=qT[:, :, i * 512 : (i + 1) * 512],
                )

            def xk_quarter(i):
                nc.sync.dma_start(
                    out=x_k[:, :, i * 512 : (i + 1) * 512],
                    in_=kT[:, :, i * 512 : (i + 1) * 512],
                )

            def xv_quarter(i):
                nc.sync.dma_start(
                    out=x_v[:, :, i * 512 : (i + 1) * 512],
                    in_=vT8[:, :, i * 512 : (i + 1) * 512],
                )

            nc.sync.dma_start(out=wk_sb, in_=wkT[:, :, :])
            xk_quarter(0)
            xq_quarter(0)
            issue_gates(0)
            xk_quarter(1)
            issue_gates(1)
            xk_quarter(2)
            issue_gates(2)
            xk_quarter(3)
            issue_gates(3)
            xv_quarter(0)
            xq_quarter(1)
            issue_gates(4)
            xv_quarter(1)
            issue_gates(5)
            xq_quarter(2)
            xv_quarter(2)
            issue_gates(6)
            xq_quarter(3)
            xv_quarter(3)

            bias_tiles = {}
            if use_bq:
                bq_sb = ws.tile([128, 2], F32, tag="bq")
                nc.sync.dma_start(out=bq_sb, in_=bqP[:, :])
                bias_tiles["q"] = bq_sb
            if use_bk:
                bk_sb = ws.tile([128, 2], F32, tag="bk")
                nc.sync.dma_start(out=bk_sb, in_=bkP[:, :])
                bias_tiles["k"] = bk_sb
            if use_bv:
                bv_sb = ws.tile([128, MPC], F32, tag="bv")
                nc.sync.dma_start(out=bv_sb, in_=bvR.ap().to_broadcast([128, MPC]))
                bias_tiles["v"] = bv_sb

            # ---- PE warm-up on a memset tile (no DMA dependency) ----
            wtile = ws.tile([128, 512], BF16, tag="wt")
            nc.vector.memset(wtile, 0.5)
            warm = ps_s.tile([128, 2 * 512], F32, tag="s", name="warm")
            for i in range(N_WARM):
                nc.tensor.matmul(
                    warm[:, 0:512],
                    lhsT=wtile[:, 0:128],
                    rhs=wtile,
                    start=True,
                    stop=True,
                    skip_group_check=True,
                )

            # ---- projection granules (512 cols, emitted in 2 halves so
            # they interleave with S-matmuls without delaying them) ----
            pj_state = {}

            def emit_qk_half(name, x_sb, w_sb, dst, mt, g4, half):
                """Half a [128, 512] granule: kc 4*half..4*half+3."""
                key = (name, mt, g4)
                if half == 0:
                    pj_state[key] = ps_s.tile(
                        [128, 512], F32, tag="s", name=f"pj_{name}{mt}{g4}"
                    )
                ps = pj_state[key]
                lo = g4 * 512
                for kc in range(4 * half, 4 * half + 4):
                    nc.tensor.matmul(
                        ps,
                        lhsT=w_sb[:, kc, mt * 128 : (mt + 1) * 128],
                        rhs=x_sb[:, kc, lo : lo + 512],
                        start=(kc == 0),
                        stop=(kc == NKC - 1),
                    )
                if half == 1:
                    pj_state.pop(key)
                    if name in bias_tiles:
                        nc.vector.tensor_scalar_add(
                            out=dst[:, mt, lo : lo + 512],
                            in0=ps,
                            scalar1=bias_tiles[name][:, mt : mt + 1],
                        )
                    else:
                        nc.scalar.copy(out=dst[:, mt, lo : lo + 512], in_=ps)

            def emit_v_lkt(lkt):
                """DoubleRow fp8 V-projection for one lk tile."""
                ps = ps_s.tile([128, MPC], F32, tag="s", name=f"pj_v{lkt}")
                for kp in range(NKC // 2):
                    nc.tensor.matmul(
                        ps,
                        lhsT=x_v[:, 2 * kp : 2 * kp + 2, lkt * 128 : (lkt + 1) * 128],
                        rhs=wv_sb[:, 2 * kp : 2 * kp + 2, :],
                        start=(kp == 0),
                        stop=(kp == NKC // 2 - 1),
                        perf_mode=DR,
                    )
                psr = ps.rearrange("p (h d) -> p h d", h=HPC)
                if "v" in bias_tiles:
                    nc.vector.tensor_add(
                        out=Vaug[:, lkt, :, 0:64],
                        in0=psr,
                        in1=bias_tiles["v"].rearrange("p (h d) -> p h d", h=HPC),
                    )
                else:
                    nc.scalar.copy(out=Vaug[:, lkt, :, 0:64], in_=psr)

            # K lk 0:512 and Q qc0 for heads 0,1 (mt=0) up front
            for half in range(2):
                emit_qk_half("k", x_k, wk_sb, KT, 0, 0, half)
            for half in range(2):
                emit_qk_half("q", x_q, wq_sb, QT, 0, 0, half)

            # ---- global software-pipelined tick stream ----
            o_tiles = {}
            tmp_tiles = {}
            p_tiles = {}

            def front(t):
                blk, k = t // NLKT, t % NLKT
                pr, qc = blk // 4, blk % 4
                j = t // 4
                if t % 4 == 0 and t // 4 + 7 < NG // 4:
                    issue_gates(t // 4 + 7)
                g_sb = gate_slabs[t // 4][:, t % 4, :]
                if t % 4 == 3:
                    gate_slabs.pop(t // 4)
                if t % 4 == 0:
                    tmp_tiles[j] = tp.tile(
                        [128, 4, 2 * QC], BF16, tag="tmp", name=f"tmp{j}"
                    )
                tmp2 = tmp_tiles[j]
                s_w = ps_s.tile([128, 2 * QC], F32, tag="s", name=f"s_{t}")
                for hp in range(2):
                    nc.tensor.matmul(
                        s_w[:, hp * QC : (hp + 1) * QC],
                        lhsT=KT[
                            hp * 64 : hp * 64 + 64, pr, k * 128 : (k + 1) * 128
                        ],
                        rhs=QT[
                            hp * 64 : hp * 64 + 64, pr, qc * QC : (qc + 1) * QC
                        ],
                        start=True,
                        stop=True,
                    )
                nc.vector.tensor_mul(out=tmp2[:, t % 4, :], in0=s_w, in1=g_sb)
                if t % 4 == 3:
                    p2 = pp.tile([128, 4, 2 * QC], FP8E5, tag="p", name=f"p{j}")
                    nc.scalar.activation(
                        out=p2, in_=tmp_tiles.pop(j), func=AF.Exp,
                        bias=ebias, scale=1.0 / 255.0,
                    )
                    p_tiles[j] = p2
                    if masked:
                        for par in range(4):
                            tt = 4 * j + par
                            kk = tt % NLKT
                            qc2 = (tt // NLKT) % 4
                            mb_sb = gp.tile([128, QC], BF16, tag="mb")
                            nc.sync.dma_start(
                                out=mb_sb,
                                in_=mbT[
                                    kk * 128 : (kk + 1) * 128,
                                    qc2 * QC : (qc2 + 1) * QC,
                                ],
                            )
                            for hp in range(2):
                                nc.vector.tensor_mul(
                                    out=p2[:, par, hp * QC : (hp + 1) * QC],
                                    in0=p2[:, par, hp * QC : (hp + 1) * QC],
                                    in1=mb_sb,
                                )

            def back_pair(p):
                """O accumulation for ticks (2p, 2p+1) via one DoubleRow
                matmul per head."""
                blk, pk = p // 8, p % 8
                pr, qc = blk // 4, blk % 4
                if pk == 0:
                    o_tiles[blk] = {
                        hp: ps_o.tile(
                            [128, QC], F32, tag="o", name=f"o_{blk}_{hp}"
                        )
                        for hp in range(2)
                    }
                j = p // 2
                idx = 2 * (p % 2)
                p2 = p_tiles[j]
                k0 = 2 * pk
                for hp in range(2):
                    nc.tensor.matmul(
                        o_tiles[blk][hp],
                        lhsT=Vaug[:, k0 : k0 + 2, 2 * pr + hp, :],
                        rhs=p2[:, idx : idx + 2, hp * QC : (hp + 1) * QC],
                        start=(pk == 0),
                        stop=(pk == 7),
                        perf_mode=DR,
                    )
                if p % 2 == 1:
                    p_tiles.pop(j)
                if pk == 7:
                    OUa = opl.tile([65, 2, QC], BF16, tag="ou")
                    for hp in range(2):
                        nc.scalar.copy(
                            out=OUa[:, hp, :], in_=o_tiles[blk][hp][0:65, :]
                        )
                    nc.gpsimd.dma_start(out=oU[pr, qc, :, :, :], in_=OUa)

            # filler schedule: tick -> (kind, mt/lkt, g4, half), placed
            # shortly before their first consuming tick (never after —
            # the in-order PE queue would deadlock on the copy).
            FILL = {}
            for i, t in enumerate((0, 1, 6, 7, 8, 9)):  # K mt0 g1..3
                FILL[t] = ("k", 0, 1 + i // 2, i % 2)
            FILL[10] = ("q", 0, 1, 0)
            FILL[11] = ("q", 0, 1, 1)
            # V-projs every other (even) tick; V_j done by back-pair use
            # at t ~ 28+j
            for j in range(16):
                FILL[12 + 2 * j] = ("v", j, 0, 0)
            FILL[29] = ("q", 0, 2, 0)  # before front(32) reads qc2
            FILL[31] = ("q", 0, 2, 1)
            FILL[33] = ("q", 0, 3, 0)
            FILL[35] = ("q", 0, 3, 1)
            for i in range(8):  # K mt1, odd ticks, before front(64)
                FILL[45 + 2 * i] = ("k", 1, i // 2, i % 2)
            for i in range(4):  # Q mt1 qc0/qc1 before front(64)/front(80)
                FILL[61 + 2 * i] = ("q", 1, i // 2, i % 2)
            for i in range(4):  # Q mt1 qc2/qc3 late (front(96)/front(112))
                FILL[85 + 2 * i] = ("q", 1, 2 + i // 2, i % 2)

            def spread_work(t):
                f = FILL.get(t)
                if f is None:
                    return
                kind, a, g4, half = f
                if kind == "k":
                    emit_qk_half("k", x_k, wk_sb, KT, a, g4, half)
                elif kind == "q":
                    emit_qk_half("q", x_q, wq_sb, QT, a, g4, half)
                else:
                    emit_v_lkt(a)

            next_pair = 0
            for t in range(NG):
                front(t)
                spread_work(t)
                while next_pair < NG // 2 and 2 * next_pair + 1 <= t - _lag(t):
                    back_pair(next_pair)
                    next_pair += 1
            while next_pair < NG // 2:
                back_pair(next_pair)
                next_pair += 1

    nc.finalize()
    return nc


C_O = 16.0  # normalized O scaled x16 into fp8e4 for L2
C_WO = 64.0  # wo scaled x64 into fp8e4
L2_DESCALE = 1.0 / (C_O * C_WO)


def build_l2(use_bo: bool, use_gamma: bool, use_beta: bool):
    nc = bacc.Bacc("TRN2", target_bir_lowering=False)

    oTf = nc.declare_dram_parameter("oTf", [128, NKC, CH], FP8E4, isOutput=False)
    woTs = nc.declare_dram_parameter("woTs", [128, NKC, D], FP8E4, isOutput=False)
    qres = nc.declare_dram_parameter("qres", [4, 128, D], BF16, isOutput=False)
    if use_bo:
        boR = nc.declare_dram_parameter("boR", [1, D], F32, isOutput=False)
    if use_gamma:
        gaR = nc.declare_dram_parameter("gaR", [1, D], F32, isOutput=False)
    if use_beta:
        beR = nc.declare_dram_parameter("beR", [1, D], F32, isOutput=False)
    yout = nc.declare_dram_parameter("yout", [4, 128, D], BF16, isOutput=True)

    with tile.TileContext(nc) as tc:
        with (
            tc.tile_pool(name="ins", bufs=1) as ins,
            tc.tile_pool(name="res", bufs=4) as res,
            tc.tile_pool(name="xb", bufs=4) as xb,
            tc.tile_pool(name="st", bufs=4) as st,
            tc.tile_pool(name="ps", bufs=8, space="PSUM") as psp,
        ):
            oT_sb = ins.tile([128, NKC, CH], FP8E4, tag="ot")
            wo_sb = ins.tile([128, NKC, D], FP8E4, tag="wo")
            # interleave kc-pairs of wo/oT so phase-A matmuls start early
            for h in range(4):
                nc.sync.dma_start(
                    out=wo_sb[:, 2 * h : 2 * h + 2, :],
                    in_=woTs[:, 2 * h : 2 * h + 2, :],
                )
                nc.sync.dma_start(
                    out=oT_sb[:, 2 * h : 2 * h + 2, :],
                    in_=oTf[:, 2 * h : 2 * h + 2, :],
                )
            eps_sb = ins.tile([128, 1], F32, tag="eps")
            nc.vector.memset(eps_sb, EPS)
            bo_sb = ga_sb = be_sb = None
            if use_bo:
                bo_sb = ins.tile([128, D], F32, tag="bo")
                nc.sync.dma_start(out=bo_sb, in_=boR.ap().to_broadcast([128, D]))
            if use_gamma:
                ga_sb = ins.tile([128, D], F32, tag="ga")
                nc.sync.dma_start(out=ga_sb, in_=gaR.ap().to_broadcast([128, D]))
            if use_beta:
                be_sb = ins.tile([128, D], F32, tag="be")
                nc.sync.dma_start(out=be_sb, in_=beR.ap().to_broadcast([128, D]))

            q_all = res.tile([128, 4, D], BF16, tag="q")
            for mm in range(4):
                nc.gpsimd.dma_start(out=q_all[:, mm, :], in_=qres[mm, :, :])

            # warm on a memset tile: no DMA dependency, PE ramps during load
            wtile = ins.tile([128, 512], BF16, tag="wt")
            nc.vector.memset(wtile, 0.5)
            warm = psp.tile([128, 512], F32, tag="mm", name="warm")
            for i in range(14):
                nc.tensor.matmul(
                    warm,
                    lhsT=wtile[:, 0:128],
                    rhs=wtile,
                    start=True,
                    stop=True,
                    skip_group_check=True,
                )

            fused_ln = bo_sb is None
            ps_mn = {
                (m, n): psp.tile([128, 512], F32, tag="mm", name=f"mm{m}{n}")
                for m in range(4)
                for n in range(2)
            }
            # phase A: kc-pairs 0..2 for all (m, n) — streams behind the DMA
            for kp in range(NKC // 2 - 1):
                for m in range(4):
                    for n in range(2):
                        nc.tensor.matmul(
                            ps_mn[(m, n)],
                            lhsT=oT_sb[:, 2 * kp : 2 * kp + 2, m * 128 : (m + 1) * 128],
                            rhs=wo_sb[:, 2 * kp : 2 * kp + 2, n * 512 : (n + 1) * 512],
                            start=(kp == 0),
                            stop=False,
                            perf_mode=DR,
                        )

            # phase B: per m, final kc-pair + LN chain (staggered tails)
            for m in range(4):
                for n in range(2):
                    nc.tensor.matmul(
                        ps_mn[(m, n)],
                        lhsT=oT_sb[:, NKC - 2 : NKC, m * 128 : (m + 1) * 128],
                        rhs=wo_sb[:, NKC - 2 : NKC, n * 512 : (n + 1) * 512],
                        start=False,
                        stop=True,
                        perf_mode=DR,
                    )
                q_sb = q_all[:, m, :]
                x = xb.tile([128, D], F32, tag="x")
                accs = st.tile([128, 2], F32, tag="accs")
                for n in range(2):
                    ps = ps_mn.pop((m, n))
                    if fused_ln:
                        nc.vector.scalar_tensor_tensor(
                            out=x[:, n * 512 : (n + 1) * 512],
                            in0=ps,
                            scalar=L2_DESCALE,
                            in1=q_sb[:, n * 512 : (n + 1) * 512],
                            op0=mybir.AluOpType.mult,
                            op1=mybir.AluOpType.add,
                            accum_out=accs[:, n : n + 1],
                        )
                    else:
                        nc.vector.scalar_tensor_tensor(
                            out=x[:, n * 512 : (n + 1) * 512],
                            in0=ps,
                            scalar=L2_DESCALE,
                            in1=q_sb[:, n * 512 : (n + 1) * 512],
                            op0=mybir.AluOpType.mult,
                            op1=mybir.AluOpType.add,
                        )
                if fused_ln:
                    scr = xb.tile([128, D], F32, tag="scr")
                    ssq = st.tile([128, 1], F32, tag="ssq")
                    nc.scalar.activation(
                        out=scr, in_=x, func=AF.Square, accum_out=ssq
                    )
                    mu = st.tile([128, 1], F32, tag="mu")
                    nc.vector.tensor_scalar(
                        out=mu,
                        in0=accs[:, 0:1],
                        scalar1=accs[:, 1:2],
                        scalar2=1.0 / D,
                        op0=mybir.AluOpType.add,
                        op1=mybir.AluOpType.mult,
                    )
                    musq = st.tile([128, 1], F32, tag="musq")
                    nc.vector.tensor_mul(out=musq, in0=mu, in1=mu)
                    var = st.tile([128, 1], F32, tag="var")
                    nc.vector.tensor_scalar(
                        out=var,
                        in0=ssq,
                        scalar1=1.0 / D,
                        scalar2=musq,
                        op0=mybir.AluOpType.mult,
                        op1=mybir.AluOpType.subtract,
                    )
                    std = st.tile([128, 1], F32, tag="std")
                    nc.scalar.activation(
                        out=std, in_=var, func=AF.Sqrt, bias=eps_sb, scale=1.0
                    )
                else:
                    if bo_sb is not None:
                        nc.vector.tensor_add(out=x, in0=x, in1=bo_sb)
                    stats = st.tile([128, 2, 6], F32, tag="stats")
                    for hh in range(2):
                        nc.vector.bn_stats(
                            out=stats[:, hh, :],
                            in_=x[:, hh * 512 : (hh + 1) * 512],
                        )
                    mv = st.tile([128, 2], F32, tag="mv")
                    nc.vector.bn_aggr(out=mv, in_=stats)
                    mu = mv[:, 0:1]
                    std = st.tile([128, 1], F32, tag="std")
                    nc.scalar.activation(
                        out=std, in_=mv[:, 1:2], func=AF.Sqrt, bias=eps_sb, scale=1.0
                    )
                rstd = st.tile([128, 1], F32, tag="rstd")
                nc.vector.reciprocal(out=rstd, in_=std)
                y = xb.tile([128, D], BF16, tag="y")
                nc.vector.tensor_scalar(
                    out=y,
                    in0=x,
                    scalar1=mu,
                    scalar2=rstd,
                    op0=mybir.AluOpType.subtract,
                    op1=mybir.AluOpType.mult,
                )
                if ga_sb is not None:
                    nc.vector.tensor_mul(out=y, in0=y, in1=ga_sb)
                if be_sb is not None:
                    nc.vector.tensor_add(out=y, in0=y, in1=be_sb)
                nc.gpsimd.dma_start(out=yout[m, :, :], in_=y)

    nc.finalize()
    return nc


_L1_CACHE = {}
_L2_CACHE = {}
LAST_RUNS = []  # (tag, nc, in_maps) of the most recent kernel() call, for profiling


def kernel(
    q, k, v, k_gate, mask, wq, bq, wk, bk, wv, bv, wo, bo, gamma, beta
):
    q = np.asarray(q, np.float32)
    k = np.asarray(k, np.float32)
    v = np.asarray(v, np.float32)
    k_gate = np.asarray(k_gate, np.float32)
    mask = np.asarray(mask)
    wq = np.asarray(wq, np.float32)
    wk = np.asarray(wk, np.float32)
    wv = np.asarray(wv, np.float32)
    wo = np.asarray(wo, np.float32)
    bq = np.asarray(bq, np.float32)
    bk = np.asarray(bk, np.float32)
    bv = np.asarray(bv, np.float32)
    bo = np.asarray(bo, np.float32)
    gamma = np.asarray(gamma, np.float32)
    beta = np.asarray(beta, np.float32)

    masked = bool(mask.any())
    use_bq = bool(np.any(bq))
    use_bk = bool(np.any(bk))
    use_bv = bool(np.any(bv))
    use_bo = bool(np.any(bo))
    use_gamma = bool(np.any(gamma != 1.0))
    use_beta = bool(np.any(beta))

    temp = float(np.float32(np.power(DK, 0.5)))

    key1 = (masked, use_bq, use_bk, use_bv)
    if key1 not in _L1_CACHE:
        _L1_CACHE[key1] = build_l1(*key1)
    nc1 = _L1_CACHE[key1]

    # ---- stage launch-1 inputs ----
    xT = {}
    for b in range(B):
        xT[("q", b)] = _bf(_kc_layout(q[b].T))
        xT[("k", b)] = _bf(_kc_layout(k[b].T))
        xT[("v", b)] = _e4(_kc_layout(v[b].T))
    wts = {}
    for hg in range(4):
        sl = slice(hg * MPC, (hg + 1) * MPC)
        wts[("q", hg)] = _bf(_kc_layout(wq[sl].T / temp))
        wts[("k", hg)] = _bf(_kc_layout(wk[sl].T))
        wts[("v", hg)] = _e4(_kc_layout(wv[sl].T * C_V))

    in_maps = []
    for c in range(NCORE):
        b, hg = c // 4, c % 4
        hsl = slice(hg * HPC, (hg + 1) * HPC)
        # gate pack: k_gate[b] is [head, lq, lk];
        # gPK[pr, qc, lkt, p, hp*512 + i] = g[2pr+hp, qc*512+i, lkt*128+p]
        gh = k_gate[b, hsl]  # [4, 2048, 2048]  (head, lq, lk)
        gr = gh.reshape(2, 2, 4, QC, NLKT, 128)  # pr, hp, qc, i, lkt, p
        gPK = np.ascontiguousarray(
            np.rint(
                gr.transpose(0, 2, 4, 5, 1, 3).reshape(2, 4, NLKT, 128, 2 * QC)
                * 255.0
            )
            .reshape(2, 4, 4, 4, 128, 2 * QC)
            .transpose(0, 1, 2, 4, 3, 5)
            .reshape(2, 4, 4, 128, 4 * 2 * QC)
        ).astype(np.uint8)
        m = {
            "qT": xT[("q", b)],
            "kT": xT[("k", b)],
            "vT8": xT[("v", b)],
            "wqT": wts[("q", hg)],
            "wkT": wts[("k", hg)],
            "wvT8": wts[("v", hg)],
            "gPK": gPK,
        }
        if use_bq:
            m["bqP"] = np.ascontiguousarray(
                (bq[hg * MPC : (hg + 1) * MPC] / temp).reshape(2, 128).T
            )
        if use_bk:
            m["bkP"] = np.ascontiguousarray(
                bk[hg * MPC : (hg + 1) * MPC].reshape(2, 128).T
            )
        if use_bv:
            m["bvR"] = (bv[hg * MPC : (hg + 1) * MPC] * C_V).reshape(1, MPC).copy()
        if masked:
            m["mbT"] = _bf((~mask[b]).astype(np.float32).T)
        in_maps.append(m)

    LAST_RUNS.clear()
    LAST_RUNS.append(("L1", nc1, in_maps))
    res1 = run_bass_kernel_spmd(nc1, in_maps, list(range(NCORE)))

    # assemble O_un^T per batch, normalize on host (O rows carry C_V)
    OTb = np.empty((B, H * DV, L), np.float32)
    DENb = np.empty((B, H, L), np.float32)
    for b in range(B):
        for hg in range(4):
            r = res1.results[b * 4 + hg]["oU"].astype(np.float32)
            # r: [pr, qc, 65, hp, QC]
            for pr in range(2):
                for hp in range(2):
                    h = hg * 4 + 2 * pr + hp
                    blk = r[pr, :, :, hp, :]  # [qc, 65, QC]
                    OTb[b, h * 64 : (h + 1) * 64, :] = np.concatenate(
                        [blk[qc, :64] for qc in range(4)], axis=1
                    )
                    DENb[b, h, :] = blk[:, 64, :].reshape(L)
    # normalize on host: rows h*64:(h+1)*64 divided by den[h] * C_V
    rd = 1.0 / (DENb * C_V)  # [B, H, L]
    OTb *= np.repeat(rd, DV, axis=1)

    key2 = (use_bo, use_gamma, use_beta)
    if key2 not in _L2_CACHE:
        _L2_CACHE[key2] = build_l2(*key2)
    nc2 = _L2_CACHE[key2]

    woTs = _e4(_kc_layout(wo.T) * C_WO)
    in_maps2 = []
    for c in range(NCORE):
        b, rchunk = c // 4, c % 4
        rows = slice(rchunk * CH, (rchunk + 1) * CH)
        otf = OTb[b][:, rows]  # [1024, 512] normalized
        m = {
            "oTf": _e4(otf.reshape(NKC, 128, CH).transpose(1, 0, 2) * C_O),
            "woTs": woTs,
            "qres": _bf(q[b, rows].reshape(4, 128, D)),
        }
        if use_bo:
            m["boR"] = bo.reshape(1, D).copy()
        if use_gamma:
            m["gaR"] = gamma.reshape(1, D).copy()
        if use_beta:
            m["beR"] = beta.reshape(1, D).copy()
        in_maps2.append(m)

    LAST_RUNS.append(("L2", nc2, in_maps2))
    res2 = run_bass_kernel_spmd(nc2, in_maps2, list(range(NCORE)))

    out = np.empty((B, L, D), np.float32)
    for c in range(NCORE):
        b, rchunk = c // 4, c % 4
        out[b, rchunk * CH : (rchunk + 1) * CH] = (
            res2.results[c]["yout"].astype(np.float32).reshape(CH, D)
        )
    return out


# revision 35
# speedup vs baseline: 1.0051x; 1.0051x over previous
"""Trainium2 Bass kernel for gated multi-head attention + residual + LayerNorm.

Problem (nn_CNP_5669356834854):
    B=2, L=2048, D=1024, H=16, DK=DV=64
    Q = q@wq.T+bq; K = k@wk.T+bk; V = v@wv.T+bv   (per-head split)
    attn = softmax((Q K^T / sqrt(DK)) * k_gate  [masked])
    out = LayerNorm(attn @ V @ wo.T + bo + q)

Sharding: 8 cores = (batch b) x (head-group hg, 4 heads).  L1 computes
UNNORMALIZED per-head attention outputs O^T plus softmax denominators
(ones-augmented V).  The host normalizes O (cheap elementwise) while
resharding; L2 shards (batch, 512-row chunk): output projection +
residual + LayerNorm.

L1 v2 (vs baseline): DVE gate-multiply is the roofline (16.8M elem/core
@ ~1.04ns).  Changes to pull the other engines below it:
  - gate staged as uint8 (g*255); the 1/255 folds into the ACT exp
    scale.  Halves gate DMA (33.5 -> 16.8 MB/core) at ~0.1% error.
  - softmax weights P written by ACT exp directly as fp8e5 (e5m2 range
    covers the e^17 logit spread; EXP_BIAS=-1 keeps max p ~1.6e4).
  - V path in fp8e4: x_v / wv (x16) / Vaug storage; V-projection and
    the O accumulation run as DoubleRow fp8 matmuls (2 contraction
    tiles per instruction, 2x PE throughput).
  - finer projection granules (512 cols) + DMA queue interleaving so
    the first S-tick starts ~12us in; V-projs and mt1 projections are
    spread between early ticks; O-matmuls lag fronts by 28 ticks early
    (V arrives late) and catch up to lag 8 past tick 64.
"""

import numpy as np
import ml_dtypes

import concourse.bacc as bacc
import concourse.tile as tile
from concourse import mybir
from concourse.bass_utils import run_bass_kernel_spmd

B, L, D, H, DK, DV = 2, 2048, 1024, 16, 64, 64
EPS = 1e-5
NCORE = 8
HPC = 4  # heads per core
NKC = D // 128  # 8 contraction chunks
NLKT = 16  # lk tiles
CH = 512  # L2 row-chunk per core
MPC = HPC * DK  # 256 projected rows per core
QC = 512  # lq per block
EXP_BIAS = -1.0
C_V = 16.0  # V scaled by 16 into fp8e4; host divides it back out

F32 = mybir.dt.float32
BF16 = mybir.dt.bfloat16
FP8E4 = mybir.dt.float8e4
FP8E5 = mybir.dt.float8e5
U8 = mybir.dt.uint8
AF = mybir.ActivationFunctionType
DR = mybir.MatmulPerfMode.DoubleRow

NPBF16 = ml_dtypes.bfloat16
NPE4 = ml_dtypes.float8_e4m3

N_WARM = 8
NG = 8 * NLKT  # 128 global ticks
G_AHEAD = 8  # gate slab DMA lookahead (ticks)
OFFLOAD = (81, 93, 105, 117)  # ticks whose gate-mult runs on GpSimd


def _bf(x):
    return np.ascontiguousarray(x).astype(NPBF16)


def _e4(x):
    return np.ascontiguousarray(x).astype(NPE4)


def _kc_layout(a):
    """[D, N] -> [128, NKC, N] with row r = kc*128+p  ->  [p, kc, :]."""
    d, n = a.shape
    assert d == NKC * 128
    return np.ascontiguousarray(a.reshape(NKC, 128, n).transpose(1, 0, 2))


def _lag(t):
    return 28 if t < 64 else max(8, 28 - (t - 64))


def build_l1(masked: bool, use_bq: bool, use_bk: bool, use_bv: bool):
    nc = bacc.Bacc("TRN2", target_bir_lowering=False)

    qT = nc.declare_dram_parameter("qT", [128, NKC, L], BF16, isOutput=False)
    kT = nc.declare_dram_parameter("kT", [128, NKC, L], BF16, isOutput=False)
    vT8 = nc.declare_dram_parameter("vT8", [128, NKC, L], FP8E4, isOutput=False)
    wqT = nc.declare_dram_parameter("wqT", [128, NKC, MPC], BF16, isOutput=False)
    wkT = nc.declare_dram_parameter("wkT", [128, NKC, MPC], BF16, isOutput=False)
    wvT8 = nc.declare_dram_parameter("wvT8", [128, NKC, MPC], FP8E4, isOutput=False)
    # host-packed gate, 4 ticks per slab:
    # gPK[pr, qc, kp, p, k4, hp*512 + i] = round(255*g) for lkt = 4*kp+k4
    gPK = nc.declare_dram_parameter(
        "gPK", [2, 4, 4, 128, 4 * 2 * QC], U8, isOutput=False
    )
    if use_bq:
        bqP = nc.declare_dram_parameter("bqP", [128, 2], F32, isOutput=False)
    if use_bk:
        bkP = nc.declare_dram_parameter("bkP", [128, 2], F32, isOutput=False)
    if use_bv:
        bvR = nc.declare_dram_parameter("bvR", [1, MPC], F32, isOutput=False)
    if masked:
        mbT = nc.declare_dram_parameter("mbT", [L, L], BF16, isOutput=False)
    # unnormalized O (rows 0:64 per hp, scaled by C_V) + denominator (row 64)
    oU = nc.declare_dram_parameter("oU", [2, 4, 65, 2, QC], BF16, isOutput=True)

    with tile.TileContext(nc) as tc:
        with (
            tc.tile_pool(name="xb", bufs=2) as xb,
            tc.tile_pool(name="xv", bufs=1) as xvp,
            tc.tile_pool(name="ws", bufs=1) as ws,
            tc.tile_pool(name="qk", bufs=1) as qk,
            tc.tile_pool(name="gp", bufs=8) as gp,
            tc.tile_pool(name="tp", bufs=2) as tp,
            tc.tile_pool(name="pp", bufs=8) as pp,
            tc.tile_pool(name="op", bufs=2) as opl,
            tc.tile_pool(name="ps_s", bufs=3, space="PSUM") as ps_s,
            tc.tile_pool(name="ps_o", bufs=2, space="PSUM") as ps_o,
        ):
            # ---- persistent tiles ----
            wk_sb = ws.tile([128, NKC, MPC], BF16, tag="wk")
            wq_sb = ws.tile([128, NKC, MPC], BF16, tag="wq")
            wv_sb = ws.tile([128, NKC, MPC], FP8E4, tag="wv")
            x_k = xb.tile([128, NKC, L], BF16, tag="x", name="x_k")
            x_q = xb.tile([128, NKC, L], BF16, tag="x", name="x_q")
            x_v = xvp.tile([128, NKC, L], FP8E4, tag="xv")
            QT = qk.tile([128, 2, L], BF16, tag="qt")
            KT = qk.tile([128, 2, L], BF16, tag="kt")
            Vaug = qk.tile([128, NLKT, HPC, 128], FP8E4, tag="va")
            nc.vector.memset(Vaug[:, :, :, 64:128], 1.0)
            ebias = ws.tile([128, 1], F32, tag="eb")
            nc.vector.memset(ebias, EXP_BIAS)

            # ---- DMA: gpsimd (SWDGE) queue: weights wq, wv, later oU ----
            nc.gpsimd.dma_start(out=wq_sb, in_=wqT[:, :, :])
            nc.gpsimd.dma_start(out=wv_sb, in_=wvT8[:, :, :])

            # ---- DMA: sync (HWDGE) queue, interleaved for earliest start --
            gate_slabs = {}  # slab group sg -> tile [128, 4, 1024]

            def issue_gates(sg):
                """One DMA for ticks 4sg .. 4sg+3 (one (pr,qc,kp) slab)."""
                blk, kp = (4 * sg) // NLKT, ((4 * sg) % NLKT) // 4
                pr, qc = blk // 4, blk % 4
                g = gp.tile([128, 4, 2 * QC], U8, tag="g", name=f"g{sg}")
                nc.sync.dma_start(out=g, in_=gPK[pr, qc, kp, :, :])
                gate_slabs[sg] = g

            def xq_quarter(i):
                nc.sync.dma_start(
                    out=x_q[:, :, i * 512 : (i + 1) * 512],
                    in_=qT[:, :, i * 512 : (i + 1) * 512],
                )

            def xk_quarter(i):
                nc.sync.dma_start(
                    out=x_k[:, :, i * 512 : (i + 1) * 512],
                    in_=kT[:, :, i * 512 : (i + 1) * 512],
                )

            def xv_quarter(i):
                nc.sync.dma_start(
                    out=x_v[:, :, i * 512 : (i + 1) * 512],
                    in_=vT8[:, :, i * 512 : (i + 1) * 512],
                )

            nc.sync.dma_start(out=wk_sb, in_=wkT[:, :, :])
            xk_quarter(0)
            xq_quarter(0)
            issue_gates(0)
            xk_quarter(1)
            issue_gates(1)
            xk_quarter(2)
            issue_gates(2)
            xk_quarter(3)
            issue_gates(3)
            xv_quarter(0)
            xq_quarter(1)
            issue_gates(4)
            xv_quarter(1)
            issue_gates(5)
            xq_quarter(2)
            xv_quarter(2)
            issue_gates(6)
            xq_quarter(3)
            xv_quarter(3)

            bias_tiles = {}
            if use_bq:
                bq_sb = ws.tile([128, 2], F32, tag="bq")
                nc.sync.dma_start(out=bq_sb, in_=bqP[:, :])
                bias_tiles["q"] = bq_sb
            if use_bk:
                bk_sb = ws.tile([128, 2], F32, tag="bk")
                nc.sync.dma_start(out=bk_sb, in_=bkP[:, :])
                bias_tiles["k"] = bk_sb
            if use_bv:
                bv_sb = ws.tile([128, MPC], F32, tag="bv")
                nc.sync.dma_start(out=bv_sb, in_=bvR.ap().to_broadcast([128, MPC]))
                bias_tiles["v"] = bv_sb

            # ---- PE warm-up on a memset tile (no DMA dependency) ----
            wtile = ws.tile([128, 512], BF16, tag="wt")
            nc.vector.memset(wtile, 0.5)
            warm = ps_s.tile([128, 2 * 512], F32, tag="s", name="warm")
            for i in range(N_WARM):
                nc.tensor.matmul(
                    warm[:, 0:512],
                    lhsT=wtile[:, 0:128],
                    rhs=wtile,
                    start=True,
                    stop=True,
                    skip_group_check=True,
                )

            # ---- projection granules (512 cols, emitted in 2 halves so
            # they interleave with S-matmuls without delaying them) ----
            pj_state = {}

            def emit_qk_half(name, x_sb, w_sb, dst, mt, g4, half):
                """Half a [128, 512] granule: kc 4*half..4*half+3."""
                key = (name, mt, g4)
                if half == 0:
                    pj_state[key] = ps_s.tile(
                        [128, 512], F32, tag="s", name=f"pj_{name}{mt}{g4}"
                    )
                ps = pj_state[key]
                lo = g4 * 512
                for kc in range(4 * half, 4 * half + 4):
                    nc.tensor.matmul(
                        ps,
                        lhsT=w_sb[:, kc, mt * 128 : (mt + 1) * 128],
                        rhs=x_sb[:, kc, lo : lo + 512],
                        start=(kc == 0),
                        stop=(kc == NKC - 1),
                    )
                if half == 1:
                    pj_state.pop(key)
                    if name in bias_tiles:
                        nc.vector.tensor_scalar_add(
                            out=dst[:, mt, lo : lo + 512],
                            in0=ps,
                            scalar1=bias_tiles[name][:, mt : mt + 1],
                        )
                    else:
                        nc.scalar.copy(out=dst[:, mt, lo : lo + 512], in_=ps)

            def emit_v_lkt(lkt):
                """DoubleRow fp8 V-projection for one lk tile."""
                ps = ps_s.tile([128, MPC], F32, tag="s", name=f"pj_v{lkt}")
                for kp in range(NKC // 2):
                    nc.tensor.matmul(
                        ps,
                        lhsT=x_v[:, 2 * kp : 2 * kp + 2, lkt * 128 : (lkt + 1) * 128],
                        rhs=wv_sb[:, 2 * kp : 2 * kp + 2, :],
                        start=(kp == 0),
                        stop=(kp == NKC // 2 - 1),
                        perf_mode=DR,
                    )
                psr = ps.rearrange("p (h d) -> p h d", h=HPC)
                if "v" in bias_tiles:
                    nc.vector.tensor_add(
                        out=Vaug[:, lkt, :, 0:64],
                        in0=psr,
                        in1=bias_tiles["v"].rearrange("p (h d) -> p h d", h=HPC),
                    )
                else:
                    nc.scalar.copy(out=Vaug[:, lkt, :, 0:64], in_=psr)

            # K lk 0:512 and Q qc0 for heads 0,1 (mt=0) up front
            for half in range(2):
                emit_qk_half("k", x_k, wk_sb, KT, 0, 0, half)
            for half in range(2):
                emit_qk_half("q", x_q, wq_sb, QT, 0, 0, half)

            # ---- global software-pipelined tick stream ----
            o_tiles = {}
            tmp_tiles = {}
            p_tiles = {}

            def front(t):
                blk, k = t // NLKT, t % NLKT
                pr, qc = blk // 4, blk % 4
                j = t // 4
                if t % 4 == 0 and t // 4 + 7 < NG // 4:
                    issue_gates(t // 4 + 7)
                g_sb = gate_slabs[t // 4][:, t % 4, :]
                if t % 4 == 3:
                    gate_slabs.pop(t // 4)
                if t % 4 == 0:
                    tmp_tiles[j] = tp.tile(
                        [128, 4, 2 * QC], BF16, tag="tmp", name=f"tmp{j}"
                    )
                tmp2 = tmp_tiles[j]
                s_w = ps_s.tile([128, 2 * QC], F32, tag="s", name=f"s_{t}")
                for hp in range(2):
                    nc.tensor.matmul(
                        s_w[:, hp * QC : (hp + 1) * QC],
                        lhsT=KT[
                            hp * 64 : hp * 64 + 64, pr, k * 128 : (k + 1) * 128
                        ],
                        rhs=QT[
                            hp * 64 : hp * 64 + 64, pr, qc * QC : (qc + 1) * QC
                        ],
                        start=True,
                        stop=True,
                    )
                if t in OFFLOAD:
                    # ACT drains PSUM to bf16, idle GpSimd does the gating
                    sB = tp.tile([128, 2 * QC], BF16, tag="sb", name=f"sb{t}")
                    nc.scalar.copy(out=sB, in_=s_w)
                    nc.gpsimd.tensor_mul(out=tmp2[:, t % 4, :], in0=sB, in1=g_sb)
                else:
                    nc.vector.tensor_mul(
                        out=tmp2[:, t % 4, :], in0=s_w, in1=g_sb
                    )
                if t % 4 == 3:
                    p2 = pp.tile([128, 4, 2 * QC], FP8E5, tag="p", name=f"p{j}")
                    tsrc = tmp_tiles.pop(j)
                    if t == NG - 1:
                        # split the last exp so the drain overlaps tick 127
                        for hh in range(2):
                            nc.scalar.activation(
                                out=p2[:, 2 * hh : 2 * hh + 2, :],
                                in_=tsrc[:, 2 * hh : 2 * hh + 2, :],
                                func=AF.Exp, bias=ebias, scale=1.0 / 255.0,
                            )
                    else:
                        nc.scalar.activation(
                            out=p2, in_=tsrc, func=AF.Exp,
                            bias=ebias, scale=1.0 / 255.0,
                        )
                    p_tiles[j] = p2
                    if masked:
                        for par in range(4):
                            tt = 4 * j + par
                            kk = tt % NLKT
                            qc2 = (tt // NLKT) % 4
                            mb_sb = gp.tile([128, QC], BF16, tag="mb")
                            nc.sync.dma_start(
                                out=mb_sb,
                                in_=mbT[
                                    kk * 128 : (kk + 1) * 128,
                                    qc2 * QC : (qc2 + 1) * QC,
                                ],
                            )
                            for hp in range(2):
                                nc.vector.tensor_mul(
                                    out=p2[:, par, hp * QC : (hp + 1) * QC],
                                    in0=p2[:, par, hp * QC : (hp + 1) * QC],
                                    in1=mb_sb,
                                )

            def back_pair(p):
                """O accumulation for ticks (2p, 2p+1) via one DoubleRow
                matmul per head."""
                blk, pk = p // 8, p % 8
                pr, qc = blk // 4, blk % 4
                if pk == 0:
                    o_tiles[blk] = {
                        hp: ps_o.tile(
                            [128, QC], F32, tag="o", name=f"o_{blk}_{hp}"
                        )
                        for hp in range(2)
                    }
                j = p // 2
                idx = 2 * (p % 2)
                p2 = p_tiles[j]
                k0 = 2 * pk
                for hp in range(2):
                    nc.tensor.matmul(
                        o_tiles[blk][hp],
                        lhsT=Vaug[:, k0 : k0 + 2, 2 * pr + hp, :],
                        rhs=p2[:, idx : idx + 2, hp * QC : (hp + 1) * QC],
                        start=(pk == 0),
                        stop=(pk == 7),
                        perf_mode=DR,
                    )
                if p % 2 == 1:
                    p_tiles.pop(j)
                if pk == 7:
                    OUa = opl.tile([65, 2, QC], BF16, tag="ou")
                    for hp in range(2):
                        nc.scalar.copy(
                            out=OUa[:, hp, :], in_=o_tiles[blk][hp][0:65, :]
                        )
                    nc.gpsimd.dma_start(out=oU[pr, qc, :, :, :], in_=OUa)

            # filler schedule: tick -> (kind, mt/lkt, g4, half), placed
            # shortly before their first consuming tick (never after —
            # the in-order PE queue would deadlock on the copy).
            FILL = {}
            for i, t in enumerate((0, 1, 6, 7, 8, 9)):  # K mt0 g1..3
                FILL[t] = ("k", 0, 1 + i // 2, i % 2)
            FILL[10] = ("q", 0, 1, 0)
            FILL[11] = ("q", 0, 1, 1)
            # V-projs every other (even) tick; V_j done by back-pair use
            # at t ~ 28+j
            for j in range(16):
                FILL[12 + 2 * j] = ("v", j, 0, 0)
            FILL[29] = ("q", 0, 2, 0)  # before front(32) reads qc2
            FILL[31] = ("q", 0, 2, 1)
            FILL[33] = ("q", 0, 3, 0)
            FILL[35] = ("q", 0, 3, 1)
            for i in range(8):  # K mt1, odd ticks, before front(64)
                FILL[45 + 2 * i] = ("k", 1, i // 2, i % 2)
            for i in range(4):  # Q mt1 qc0/qc1 before front(64)/front(80)
                FILL[61 + 2 * i] = ("q", 1, i // 2, i % 2)
            for i in range(4):  # Q mt1 qc2/qc3 late (front(96)/front(112))
                FILL[85 + 2 * i] = ("q", 1, 2 + i // 2, i % 2)

            def spread_work(t):
                f = FILL.get(t)
                if f is None:
                    return
                kind, a, g4, half = f
                if kind == "k":
                    emit_qk_half("k", x_k, wk_sb, KT, a, g4, half)
                elif kind == "q":
                    emit_qk_half("q", x_q, wq_sb, QT, a, g4, half)
                else:
                    emit_v_lkt(a)

            next_pair = 0
            for t in range(NG):
                front(t)
                spread_work(t)
                while next_pair < NG // 2 and 2 * next_pair + 1 <= t - _lag(t):
                    back_pair(next_pair)
                    next_pair += 1
            while next_pair < NG // 2:
                back_pair(next_pair)
                next_pair += 1

    nc.finalize()
    return nc


C_O = 16.0  # normalized O scaled x16 into fp8e4 for L2
C_WO = 64.0  # wo scaled x64 into fp8e4
L2_DESCALE = 1.0 / (C_O * C_WO)


def build_l2(use_bo: bool, use_gamma: bool, use_beta: bool):
    nc = bacc.Bacc("TRN2", target_bir_lowering=False)

    oTf = nc.declare_dram_parameter("oTf", [128, NKC, CH], FP8E4, isOutput=False)
    woTs = nc.declare_dram_parameter("woTs", [128, NKC, D], FP8E4, isOutput=False)
    qres = nc.declare_dram_parameter("qres", [4, 128, D], BF16, isOutput=False)
    if use_bo:
        boR = nc.declare_dram_parameter("boR", [1, D], F32, isOutput=False)
    if use_gamma:
        gaR = nc.declare_dram_parameter("gaR", [1, D], F32, isOutput=False)
    if use_beta:
        beR = nc.declare_dram_parameter("beR", [1, D], F32, isOutput=False)
    yout = nc.declare_dram_parameter("yout", [4, 128, D], BF16, isOutput=True)

    with tile.TileContext(nc) as tc:
        with (
            tc.tile_pool(name="ins", bufs=1) as ins,
            tc.tile_pool(name="res", bufs=4) as res,
            tc.tile_pool(name="xb", bufs=4) as xb,
            tc.tile_pool(name="st", bufs=4) as st,
            tc.tile_pool(name="ps", bufs=8, space="PSUM") as psp,
        ):
            oT_sb = ins.tile([128, NKC, CH], FP8E4, tag="ot")
            wo_sb = ins.tile([128, NKC, D], FP8E4, tag="wo")
            # interleave kc-pairs of wo/oT so phase-A matmuls start early
            for h in range(4):
                nc.sync.dma_start(
                    out=wo_sb[:, 2 * h : 2 * h + 2, :],
                    in_=woTs[:, 2 * h : 2 * h + 2, :],
                )
                nc.sync.dma_start(
                    out=oT_sb[:, 2 * h : 2 * h + 2, :],
                    in_=oTf[:, 2 * h : 2 * h + 2, :],
                )
            eps_sb = ins.tile([128, 1], F32, tag="eps")
            nc.vector.memset(eps_sb, EPS)
            bo_sb = ga_sb = be_sb = None
            if use_bo:
                bo_sb = ins.tile([128, D], F32, tag="bo")
                nc.sync.dma_start(out=bo_sb, in_=boR.ap().to_broadcast([128, D]))
            if use_gamma:
                ga_sb = ins.tile([128, D], F32, tag="ga")
                nc.sync.dma_start(out=ga_sb, in_=gaR.ap().to_broadcast([128, D]))
            if use_beta:
                be_sb = ins.tile([128, D], F32, tag="be")
                nc.sync.dma_start(out=be_sb, in_=beR.ap().to_broadcast([128, D]))

            q_all = res.tile([128, 4, D], BF16, tag="q")
            for mm in range(4):
                nc.gpsimd.dma_start(out=q_all[:, mm, :], in_=qres[mm, :, :])

            # warm on a memset tile: no DMA dependency, PE ramps during load
            wtile = ins.tile([128, 512], BF16, tag="wt")
            nc.vector.memset(wtile, 0.5)
            warm = psp.tile([128, 512], F32, tag="mm", name="warm")
            for i in range(14):
                nc.tensor.matmul(
                    warm,
                    lhsT=wtile[:, 0:128],
                    rhs=wtile,
                    start=True,
                    stop=True,
                    skip_group_check=True,
                )

            fused_ln = bo_sb is None
            ps_mn = {
                (m, n): psp.tile([128, 512], F32, tag="mm", name=f"mm{m}{n}")
                for m in range(4)
                for n in range(2)
            }
            # phase A: kc-pairs 0..2 for all (m, n) — streams behind the DMA
            for kp in range(NKC // 2 - 1):
                for m in range(4):
                    for n in range(2):
                        nc.tensor.matmul(
                            ps_mn[(m, n)],
                            lhsT=oT_sb[:, 2 * kp : 2 * kp + 2, m * 128 : (m + 1) * 128],
                            rhs=wo_sb[:, 2 * kp : 2 * kp + 2, n * 512 : (n + 1) * 512],
                            start=(kp == 0),
                            stop=False,
                            perf_mode=DR,
                        )

            # phase B: per m, final kc-pair + LN chain (staggered tails)
            for m in range(4):
                for n in range(2):
                    nc.tensor.matmul(
                        ps_mn[(m, n)],
                        lhsT=oT_sb[:, NKC - 2 : NKC, m * 128 : (m + 1) * 128],
                        rhs=wo_sb[:, NKC - 2 : NKC, n * 512 : (n + 1) * 512],
                        start=False,
                        stop=True,
                        perf_mode=DR,
                    )
                q_sb = q_all[:, m, :]
                x = xb.tile([128, D], F32, tag="x")
                accs = st.tile([128, 2], F32, tag="accs")
                for n in range(2):
                    ps = ps_mn.pop((m, n))
                    if fused_ln:
                        nc.vector.scalar_tensor_tensor(
                            out=x[:, n * 512 : (n + 1) * 512],
                            in0=ps,
                            scalar=L2_DESCALE,
                            in1=q_sb[:, n * 512 : (n + 1) * 512],
                            op0=mybir.AluOpType.mult,
                            op1=mybir.AluOpType.add,
                            accum_out=accs[:, n : n + 1],
                        )
                    else:
                        nc.vector.scalar_tensor_tensor(
                            out=x[:, n * 512 : (n + 1) * 512],
                            in0=ps,
                            scalar=L2_DESCALE,
                            in1=q_sb[:, n * 512 : (n + 1) * 512],
                            op0=mybir.AluOpType.mult,
                            op1=mybir.AluOpType.add,
                        )
                if fused_ln:
                    scr = xb.tile([128, D], F32, tag="scr")
                    ssq = st.tile([128, 1], F32, tag="ssq")
                    nc.scalar.activation(
                        out=scr, in_=x, func=AF.Square, accum_out=ssq
                    )
                    mu = st.tile([128, 1], F32, tag="mu")
                    nc.vector.tensor_scalar(
                        out=mu,
                        in0=accs[:, 0:1],
                        scalar1=accs[:, 1:2],
                        scalar2=1.0 / D,
                        op0=mybir.AluOpType.add,
                        op1=mybir.AluOpType.mult,
                    )
                    musq = st.tile([128, 1], F32, tag="musq")
                    nc.vector.tensor_mul(out=musq, in0=mu, in1=mu)
                    var = st.tile([128, 1], F32, tag="var")
                    nc.vector.tensor_scalar(
                        out=var,
                        in0=ssq,
                        scalar1=1.0 / D,
                        scalar2=musq,
                        op0=mybir.AluOpType.mult,
                        op1=mybir.AluOpType.subtract,
                    )
                    std = st.tile([128, 1], F32, tag="std")
                    nc.scalar.activation(
                        out=std, in_=var, func=AF.Sqrt, bias=eps_sb, scale=1.0
                    )
                else:
                    if bo_sb is not None:
                        nc.vector.tensor_add(out=x, in0=x, in1=bo_sb)
                    stats = st.tile([128, 2, 6], F32, tag="stats")
                    for hh in range(2):
                        nc.vector.bn_stats(
                            out=stats[:, hh, :],
                            in_=x[:, hh * 512 : (hh + 1) * 512],
                        )
                    mv = st.tile([128, 2], F32, tag="mv")
                    nc.vector.bn_aggr(out=mv, in_=stats)
                    mu = mv[:, 0:1]
                    std = st.tile([128, 1], F32, tag="std")
                    nc.scalar.activation(
                        out=std, in_=mv[:, 1:2], func=AF.Sqrt, bias=eps_sb, scale=1.0
                    )
                rstd = st.tile([128, 1], F32, tag="rstd")
                nc.vector.reciprocal(out=rstd, in_=std)
                y = xb.tile([128, D], BF16, tag="y")
                nc.vector.tensor_scalar(
                    out=y,
                    in0=x,
                    scalar1=mu,
                    scalar2=rstd,
                    op0=mybir.AluOpType.subtract,
                    op1=mybir.AluOpType.mult,
                )
                if ga_sb is not None:
                    nc.vector.tensor_mul(out=y, in0=y, in1=ga_sb)
                if be_sb is not None:
                    nc.vector.tensor_add(out=y, in0=y, in1=be_sb)
                nc.gpsimd.dma_start(out=yout[m, :, :], in_=y)

    nc.finalize()
    return nc


_L1_CACHE = {}
_L2_CACHE = {}
LAST_RUNS = []  # (tag, nc, in_maps) of the most recent kernel() call, for profiling


def kernel(
    q, k, v, k_gate, mask, wq, bq, wk, bk, wv, bv, wo, bo, gamma, beta
):
    q = np.asarray(q, np.float32)
    k = np.asarray(k, np.float32)
    v = np.asarray(v, np.float32)
    k_gate = np.asarray(k_gate, np.float32)
    mask = np.asarray(mask)
    wq = np.asarray(wq, np.float32)
    wk = np.asarray(wk, np.float32)
    wv = np.asarray(wv, np.float32)
    wo = np.asarray(wo, np.float32)
    bq = np.asarray(bq, np.float32)
    bk = np.asarray(bk, np.float32)
    bv = np.asarray(bv, np.float32)
    bo = np.asarray(bo, np.float32)
    gamma = np.asarray(gamma, np.float32)
    beta = np.asarray(beta, np.float32)

    masked = bool(mask.any())
    use_bq = bool(np.any(bq))
    use_bk = bool(np.any(bk))
    use_bv = bool(np.any(bv))
    use_bo = bool(np.any(bo))
    use_gamma = bool(np.any(gamma != 1.0))
    use_beta = bool(np.any(beta))

    temp = float(np.float32(np.power(DK, 0.5)))

    key1 = (masked, use_bq, use_bk, use_bv)
    if key1 not in _L1_CACHE:
        _L1_CACHE[key1] = build_l1(*key1)
    nc1 = _L1_CACHE[key1]

    # ---- stage launch-1 inputs ----
    xT = {}
    for b in range(B):
        xT[("q", b)] = _bf(_kc_layout(q[b].T))
        xT[("k", b)] = _bf(_kc_layout(k[b].T))
        xT[("v", b)] = _e4(_kc_layout(v[b].T))
    wts = {}
    for hg in range(4):
        sl = slice(hg * MPC, (hg + 1) * MPC)
        wts[("q", hg)] = _bf(_kc_layout(wq[sl].T / temp))
        wts[("k", hg)] = _bf(_kc_layout(wk[sl].T))
        wts[("v", hg)] = _e4(_kc_layout(wv[sl].T * C_V))

    in_maps = []
    for c in range(NCORE):
        b, hg = c // 4, c % 4
        hsl = slice(hg * HPC, (hg + 1) * HPC)
        # gate pack: k_gate[b] is [head, lq, lk];
        # gPK[pr, qc, lkt, p, hp*512 + i] = g[2pr+hp, qc*512+i, lkt*128+p]
        gh = k_gate[b, hsl]  # [4, 2048, 2048]  (head, lq, lk)
        gr = gh.reshape(2, 2, 4, QC, NLKT, 128)  # pr, hp, qc, i, lkt, p
        gPK = np.ascontiguousarray(
            np.rint(
                gr.transpose(0, 2, 4, 5, 1, 3).reshape(2, 4, NLKT, 128, 2 * QC)
                * 255.0
            )
            .reshape(2, 4, 4, 4, 128, 2 * QC)
            .transpose(0, 1, 2, 4, 3, 5)
            .reshape(2, 4, 4, 128, 4 * 2 * QC)
        ).astype(np.uint8)
        m = {
            "qT": xT[("q", b)],
            "kT": xT[("k", b)],
            "vT8": xT[("v", b)],
            "wqT": wts[("q", hg)],
            "wkT": wts[("k", hg)],
            "wvT8": wts[("v", hg)],
            "gPK": gPK,
        }
        if use_bq:
            m["bqP"] = np.ascontiguousarray(
                (bq[hg * MPC : (hg + 1) * MPC] / temp).reshape(2, 128).T
            )
        if use_bk:
            m["bkP"] = np.ascontiguousarray(
                bk[hg * MPC : (hg + 1) * MPC].reshape(2, 128).T
            )
        if use_bv:
            m["bvR"] = (bv[hg * MPC : (hg + 1) * MPC] * C_V).reshape(1, MPC).copy()
        if masked:
            m["mbT"] = _bf((~mask[b]).astype(np.float32).T)
        in_maps.append(m)

    LAST_RUNS.clear()
    LAST_RUNS.append(("L1", nc1, in_maps))
    res1 = run_bass_kernel_spmd(nc1, in_maps, list(range(NCORE)))

    # assemble O_un^T per batch, normalize on host (O rows carry C_V)
    OTb = np.empty((B, H * DV, L), np.float32)
    DENb = np.empty((B, H, L), np.float32)
    for b in range(B):
        for hg in range(4):
            r = res1.results[b * 4 + hg]["oU"].astype(np.float32)
            # r: [pr, qc, 65, hp, QC]
            for pr in range(2):
                for hp in range(2):
                    h = hg * 4 + 2 * pr + hp
                    blk = r[pr, :, :, hp, :]  # [qc, 65, QC]
                    OTb[b, h * 64 : (h + 1) * 64, :] = np.concatenate(
                        [blk[qc, :64] for qc in range(4)], axis=1
                    )
                    DENb[b, h, :] = blk[:, 64, :].reshape(L)
    # normalize on host: rows h*64:(h+1)*64 divided by den[h] * C_V
    rd = 1.0 / (DENb * C_V)  # [B, H, L]
    OTb *= np.repeat(rd, DV, axis=1)

    key2 = (use_bo, use_gamma, use_beta)
    if key2 not in _L2_CACHE:
        _L2_CACHE[key2] = build_l2(*key2)
    nc2 = _L2_CACHE[key2]

    woTs = _e4(_kc_layout(wo.T) * C_WO)
    in_maps2 = []
    for c in range(NCORE):
        b, rchunk = c // 4, c % 4
        rows = slice(rchunk * CH, (rchunk + 1) * CH)
        otf = OTb[b][:, rows]  # [1024, 512] normalized
        m = {
            "oTf": _e4(otf.reshape(NKC, 128, CH).transpose(1, 0, 2) * C_O),
            "woTs": woTs,
            "qres": _bf(q[b, rows].reshape(4, 128, D)),
        }
        if use_bo:
            m["boR"] = bo.reshape(1, D).copy()
        if use_gamma:
            m["gaR"] = gamma.reshape(1, D).copy()
        if use_beta:
            m["beR"] = beta.reshape(1, D).copy()
        in_maps2.append(m)

    LAST_RUNS.append(("L2", nc2, in_maps2))
    res2 = run_bass_kernel_spmd(nc2, in_maps2, list(range(NCORE)))

    out = np.empty((B, L, D), np.float32)
    for c in range(NCORE):
        b, rchunk = c // 4, c % 4
        out[b, rchunk * CH : (rchunk + 1) * CH] = (
            res2.results[c]["yout"].astype(np.float32).reshape(CH, D)
        )
    return out
